# revision 3
# baseline (speedup 1.0000x reference)
"""Trainium2 Bass kernel for nn_DiscretizedGaussian (discretized-Gaussian log-likelihood).

Computation per element (mean m, logvar lv, data x):
    idx   = rint(127.5*(x+1))                     (bin index, 0..255)
    t'    = idx - 128*m                           (sign-flipped, 128-scaled "x_sel - m")
    iv    = exp(-lv - ln(128))                    (= inv_std/128)
    u+    = (t' + (hh-127.5)) * iv  = -v_minus    (hh = 128/255)
    u-    = (t' - (hh+127.5)) * iv  = -v_plus
    z~    = (u^2 + 1/0.044715) * u    ;  T = tanh(b2 * z~),  b2 = sqrt(2/pi)*0.044715
    d     = T+ - T-  = tanh(P(v_plus)) - tanh(P(v_minus))   (odd symmetry)
    ll    = log(0.5*d + 1e-10)                    (== log(max(cdf_d, 1e-10)) for d>=0)
    out_s = sum over all elements of sample s.

Engine split per [128, 2048] block (24 blocks/core, 8 cores data-parallel over batch):
    DVE : idx chain (3x tensor_scalar), u+/u- (scalar_tensor_tensor from PSUM), z~ (STT in-place)
    Pool: squares (tensor_tensor)
    ACT : exp, 2x tanh, log (+accum_out = free per-partition reduce)
    PE  : t' = I@idx + (-128I)@m  and  d = I@T+ + (-I)@T- as accumulating matmuls;
          final cross-partition per-sample reduce via G-matmul.
"""
import sys
for _p in ("/opt/trn_rl_repo", "/opt/trn_rl_repo/concourse"):
    if _p not in sys.path:
        sys.path.insert(0, _p)

from contextlib import ExitStack
import numpy as np

import concourse.bass as bass  # noqa: F401  (registers AP machinery)
import concourse.tile as tile
from concourse import bacc, mybir
from concourse import bass_utils

F32 = mybir.dt.float32
P = 128
FB = 2048                 # free-dim block size
NBLK = 24                 # blocks per core
FREE = FB * NBLK          # 49152 free elems per partition per core
NCORE = 8
SPB = 8                   # samples per core (64 / 8)
B, C, H, W = 64, 3, 512, 512

LN128 = float(np.log(np.float64(128.0)))
HH = float(np.float64(128.0) / np.float64(255.0))
CP = float(np.float64(HH) - 127.5)
CM = float(-np.float64(HH) - 127.5)
CC = float(np.float64(1.0) / np.float64(0.044715))
B2 = float(np.float64(0.7978845608028654) * np.float64(0.044715))
MAGIC = float(2.0 ** 23)

_CACHE = {}


def _consts_np():
    I = np.eye(P, dtype=np.float32)
    w_m = (-128.0 * I).astype(np.float32)
    w_neg = (-I).astype(np.float32)
    G = np.zeros((P, SPB), np.float32)
    for k in range(P):
        G[k, k // 16] = 1.0
    bias_exp = np.full((P, 1), -LN128, np.float32)
    bias_ln = np.full((P, 1), 1e-10, np.float32)
    return np.ascontiguousarray(
        np.concatenate([I, w_m, w_neg, G, bias_exp, bias_ln], axis=1),
        dtype=np.float32,
    )  # [128, 394]


def _build():
    A = mybir.AluOpType
    AF = mybir.ActivationFunctionType
    nc = bacc.Bacc(
        "TRN2",
        target_bir_lowering=False,
        debug=False,
        enable_asserts=False,
        num_devices=NCORE,
    )
    m_in = nc.dram_tensor("m_in", [P, FREE], F32, kind="ExternalInput").ap()
    lv_in = nc.dram_tensor("lv_in", [P, FREE], F32, kind="ExternalInput").ap()
    x_in = nc.dram_tensor("x_in", [P, FREE], F32, kind="ExternalInput").ap()
    c_in = nc.dram_tensor("c_in", [P, 394], F32, kind="ExternalInput").ap()
    o_out = nc.dram_tensor("o_out", [1, SPB], F32, kind="ExternalOutput").ap()

    with tile.TileContext(nc) as tc, ExitStack() as ctx:
        pin = ctx.enter_context(tc.tile_pool(name="pin", bufs=2))
        psc = ctx.enter_context(tc.tile_pool(name="psc", bufs=4))
        piv = ctx.enter_context(tc.tile_pool(name="piv", bufs=2))
        pu = ctx.enter_context(tc.tile_pool(name="pu", bufs=3))
        psq = ctx.enter_context(tc.tile_pool(name="psq", bufs=3))
        pT = ctx.enter_context(tc.tile_pool(name="pT", bufs=3))
        pone = ctx.enter_context(tc.tile_pool(name="pone", bufs=1))
        pps_t = ctx.enter_context(tc.tile_pool(name="pps_t", bufs=2, space="PSUM"))
        pps_d = ctx.enter_context(tc.tile_pool(name="pps_d", bufs=3, space="PSUM"))
        pps_o = ctx.enter_context(tc.tile_pool(name="pps_o", bufs=1, space="PSUM"))

        consts = pone.tile([P, 394], F32, tag="consts")
        nc.sync.dma_start(consts[:], c_in[:])
        W_IDX = consts[:, 0:128]
        W_M = consts[:, 128:256]
        W_NEG = consts[:, 256:384]
        G = consts[:, 384:392]
        BIAS_EXP = consts[:, 392:393]
        BIAS_LN = consts[:, 393:394]
        partials = pone.tile([P, 4 * NBLK], F32, tag="partials")

        for b in range(NBLK):
            c0 = b * FB
            x_t = pin.tile([P, FB], F32, tag="x")
            nc.sync.dma_start(x_t[:], x_in[:, c0:c0 + FB])
            m_t = pin.tile([P, FB], F32, tag="m")
            nc.sync.dma_start(m_t[:], m_in[:, c0:c0 + FB])
            lv_t = pin.tile([P, FB], F32, tag="lv")
            nc.sync.dma_start(lv_t[:], lv_in[:, c0:c0 + FB])

            # idx = rint(127.5*(x+1)) via separate fp32-rounded ops
            w_t = psc.tile([P, FB], F32, tag="sc")
            nc.vector.tensor_scalar(w_t[:], x_t[:], 1.0, 127.5, A.add, A.mult)
            bb_t = psc.tile([P, FB], F32, tag="sc")
            nc.vector.tensor_scalar(bb_t[:], w_t[:], MAGIC, None, A.add)
            idx_t = psc.tile([P, FB], F32, tag="sc")
            nc.vector.tensor_scalar(idx_t[:], bb_t[:], MAGIC, None, A.subtract)

            iv_t = piv.tile([P, FB], F32, tag="iv")
            nc.scalar.activation(iv_t[:], lv_t[:], AF.Exp, bias=BIAS_EXP, scale=-1.0)

            up_t = pu.tile([P, FB], F32, tag="u")
            um_t = pu.tile([P, FB], F32, tag="u")
            for j in (0, 1):
                s = slice(j * 1024, (j + 1) * 1024)
                t_ps = pps_t.tile([P, 1024], F32, tag="t")
                for h in (0, 1):
                    ss = slice(j * 1024 + h * 512, j * 1024 + (h + 1) * 512)
                    ps_s = slice(h * 512, (h + 1) * 512)
                    nc.tensor.matmul(t_ps[:, ps_s], W_IDX, idx_t[:, ss],
                                     start=True, stop=False)
                    nc.tensor.matmul(t_ps[:, ps_s], W_M, m_t[:, ss],
                                     start=False, stop=True)
                nc.vector.scalar_tensor_tensor(
                    up_t[:, s], t_ps[:], CP, iv_t[:, s], A.add, A.mult)
                nc.vector.scalar_tensor_tensor(
                    um_t[:, s], t_ps[:], CM, iv_t[:, s], A.add, A.mult)

            sp_t = psq.tile([P, FB], F32, tag="s")
            nc.gpsimd.tensor_tensor(sp_t[:], up_t[:], up_t[:], A.mult)
            sm_t = psq.tile([P, FB], F32, tag="s")
            nc.gpsimd.tensor_tensor(sm_t[:], um_t[:], um_t[:], A.mult)

            # z~ = (s + CC) * u, in place over s
            nc.vector.scalar_tensor_tensor(sp_t[:], sp_t[:], CC, up_t[:], A.add, A.mult)
            nc.vector.scalar_tensor_tensor(sm_t[:], sm_t[:], CC, um_t[:], A.add, A.mult)

            Tp_t = pT.tile([P, FB], F32, tag="T")
            nc.scalar.activation(Tp_t[:], sp_t[:], AF.Tanh, scale=B2)
            Tm_t = pT.tile([P, FB], F32, tag="T")
            nc.scalar.activation(Tm_t[:], sm_t[:], AF.Tanh, scale=B2)

            for q in range(4):
                ss = slice(q * 512, (q + 1) * 512)
                d_ps = pps_d.tile([P, 512], F32, tag="d")
                nc.tensor.matmul(d_ps[:], W_IDX, Tp_t[:, ss], start=True, stop=False)
                nc.tensor.matmul(d_ps[:], W_NEG, Tm_t[:, ss], start=False, stop=True)
                nc.scalar.activation(d_ps[:], d_ps[:], AF.Ln,
                                     bias=BIAS_LN, scale=0.5,
                                     accum_out=partials[:, b * 4 + q: b * 4 + q + 1])

        part_sum = pone.tile([P, 1], F32, tag="psum1")
        nc.vector.tensor_reduce(part_sum[:], partials[:],
                                axis=mybir.AxisListType.X, op=A.add)
        out_ps = pps_o.tile([1, SPB], F32, tag="outp")
        nc.tensor.matmul(out_ps[:], part_sum[:], G, start=True, stop=True)
        out_sb = pone.tile([1, SPB], F32, tag="outs")
        nc.vector.tensor_copy(out_sb[:], out_ps[:])
        nc.sync.dma_start(o_out[:], out_sb[:])
    nc.compile()
    return nc


def _get_nc():
    if "nc" not in _CACHE:
        _CACHE["nc"] = _build()
    return _CACHE["nc"]


def _make_in_maps(mean, logvar, x):
    consts = _consts_np()
    in_maps = []
    for k in range(NCORE):
        sl = slice(k * SPB, (k + 1) * SPB)
        in_maps.append({
            "m_in": np.ascontiguousarray(mean[sl], dtype=np.float32).reshape(P, FREE),
            "lv_in": np.ascontiguousarray(logvar[sl], dtype=np.float32).reshape(P, FREE),
            "x_in": np.ascontiguousarray(x[sl], dtype=np.float32).reshape(P, FREE),
            "c_in": consts,
        })
    return in_maps


def _run(in_maps, trace=False):
    nc = _get_nc()
    return bass_utils.run_bass_kernel_spmd(
        nc, in_maps, core_ids=list(range(NCORE)), trace=trace)


def kernel(mean, logvar, x):
    assert mean.shape == (B, C, H, W), mean.shape
    res = _run(_make_in_maps(mean, logvar, x), trace=False)
    out = np.concatenate([r["o_out"].reshape(SPB) for r in res.results])
    return out.astype(np.float32)


if __name__ == "__main__":
    rng = np.random.default_rng(0)
    m = (rng.standard_normal((B, C, H, W)) * 0.1).astype(np.float32)
    lv = (rng.standard_normal((B, C, H, W)) * 0.1 - 2.0).astype(np.float32)
    xx = rng.uniform(-1.0, 1.0 - 1e-6, (B, C, H, W)).astype(np.float32)
    out = kernel(m, lv, xx)
    print("kernel out[:8]:", out[:8])
